# revision 58
# baseline (speedup 1.0000x reference)
"""Multi-Head Latent Attention (MLA) Bass kernel for Trainium2, 8 NeuronCores.

Problem: B=2, S=2048, D=2048, H=16, D_NOPE=128, D_ROPE=64, D_V=128, R_Q=1536, R_KV=512.

Sharding: core c = b*4 + g handles batch b, head group g (heads 4g..4g+3).
Compress (x -> cq/ckv/krope latents) is sequence-sharded across the 4 cores of a
batch group (each core compresses its own 512-column S-shard), then on-device
AllGathers within each batch group assemble full-S latents. Decompress,
attention and out-proj are head-sharded; each core emits a partial y which the
host sums.

Pipeline notes (v2):
- Compress matmuls run r-tile-outer so each latent r-tile completes (and its
  store DMA + gather trigger can fire) as early as possible, instead of all
  r-tiles completing together at phase end.
- Collectives merged into 3 ops (kv: 578 rows; cq: 2x 768+2 rows) to amortize
  the ~8-15us fixed per-op cost on the serial CC stream.
- DMA issue order is managed explicitly: gather-input stores are enqueued
  before the bulk decompress-weight prefetches that would otherwise sit ahead
  of them in the round-robin HWDGE queues and delay the collective triggers.
- Decompress weights (wdk/wdv/wdqn/wdqr/wproj) and gathered-latent loads are
  prefetched during compress so the Dkv/Dq phases start without DMA stalls.

Key algebraic simplifications (exact):
- RoPE uses per-head angles constant across positions, applied identically to
  q_rope and k_rope => rotations cancel in q.k, so RoPE is skipped entirely.
- RMSNorm scale rstd[s] folded post-decompress (q/k columns via broadcast tiles,
  v rows via per-partition scalars); norm weights and the V-scale 1/sqrt(H*D_V)
  folded into decompress weights on the host.
- Softmax without max subtraction: probs = exp(s)*mask; the denominator l is
  accumulated on the Vector engine (lsum += pt per key-tile) and reduced with a
  single ones-matmul per (head, chunk) instead of one matmul per key-tile.
- q_rope decompress packs head pairs into single 128-wide matmuls; attention
  reads the packed tiles at partition offsets 0/64 (PE tile_position).

All matmul operands are bf16 except small f32r helpers (rstd rows, lsum);
psum accumulation is fp32 throughout.
"""
import sys
sys.path.insert(0, '/opt/trn_rl_repo')

import numpy as np
import ml_dtypes
from contextlib import ExitStack

from concourse import bacc, tile
import concourse.mybir as mybir
from concourse.bass_utils import run_bass_kernel_spmd

f32 = mybir.dt.float32
f32r = mybir.dt.float32r
bf16 = mybir.dt.bfloat16
f8 = mybir.dt.float8e4

B, S, D = 2, 2048, 2048
H, DN, DR, DV = 16, 128, 64, 128
RQ, RKV = 1536, 512
EPS = 1e-5
HG = 4                      # heads per group
SC = 512                    # S-chunk width
NC_ = 8                     # cores
ATTN_SCALE = float(1.0 / np.sqrt(DN + DR))
Act = mybir.ActivationFunctionType
RG = [[0, 1, 2, 3], [4, 5, 6, 7]]

_CACHED_NC = None


def _build():
    nc = bacc.Bacc("TRN2", target_bir_lowering=False, debug=False, num_devices=NC_)

    xs = nc.declare_dram_parameter("xs", [D, SC], bf16, isOutput=False)
    w_cq = nc.declare_dram_parameter("w_cq", [D, RQ], bf16, isOutput=False)
    w_ckv = nc.declare_dram_parameter("w_ckv", [D, RKV], bf16, isOutput=False)
    w_kr = nc.declare_dram_parameter("w_kr", [D, DR], bf16, isOutput=False)
    w_dqn = nc.declare_dram_parameter("w_dqn", [RQ, HG * DN], bf16, isOutput=False)
    w_dqr = nc.declare_dram_parameter("w_dqr", [RQ, HG * DR], bf16, isOutput=False)
    w_dk = nc.declare_dram_parameter("w_dk", [RKV, HG * DN], bf16, isOutput=False)
    w_dv = nc.declare_dram_parameter("w_dv", [RKV, HG * DV], bf16, isOutput=False)
    w_proj = nc.declare_dram_parameter("w_proj", [HG * DV, D], bf16, isOutput=False)
    masks_in = nc.declare_dram_parameter("masks", [4, 128, SC], bf16, isOutput=False)
    eye4r_in = nc.declare_dram_parameter("eye4r", [4, 512], f32r, isOutput=False)
    eye4c_in = nc.declare_dram_parameter("eye4c", [4, 4], f32, isOutput=False)
    ones_r_in = nc.declare_dram_parameter("ones_r", [128, 128], f32r, isOutput=False)
    ones_b_in = nc.declare_dram_parameter("ones_b", [128, 1], bf16, isOutput=False)
    ones_br_in = nc.declare_dram_parameter("ones_br", [128, 1], f32r, isOutput=False)
    y_out = nc.declare_dram_parameter("y", [S, D], bf16, isOutput=True)

    with tile.TileContext(nc) as tc, ExitStack() as ctx:
        keep = ctx.enter_context(tc.tile_pool(name="keep", bufs=1))
        dram = ctx.enter_context(tc.tile_pool(name="dram", bufs=1, space="DRAM"))
        # long-lived prefetch targets: decompress weights + gathered latents
        pre = ctx.enter_context(tc.tile_pool(name="pre", bufs=1))

        ones_r = keep.tile([128, 128], f32r)
        nc.sync.dma_start(ones_r[:], ones_r_in[:])
        ones_b = keep.tile([128, 1], bf16)
        nc.sync.dma_start(ones_b[:], ones_b_in[:])
        ones_br = keep.tile([128, 1], f32r)
        nc.sync.dma_start(ones_br[:], ones_br_in[:])
        masks = keep.tile([128, 4 * SC], bf16)
        for i in range(4):
            nc.sync.dma_start(masks[:, i * SC:(i + 1) * SC], masks_in[i])
        eye4r_sb = keep.tile([4, 512], f32r)
        nc.sync.dma_start(eye4r_sb[:], eye4r_in[:])
        eye4c_sb = keep.tile([4, 4], f32)
        nc.sync.dma_start(eye4c_sb[:], eye4c_in[:])

        wdk_sb = pre.tile([128, 4 * HG * DN], bf16)     # r-tile r at cols r*512
        wdv_sb = pre.tile([128, 4 * HG * DV], bf16)
        wdqn_sb = pre.tile([128, 12 * HG * DN], bf16)   # r-tile r at cols r*512
        wdqr_sb = pre.tile([128, 12 * HG * DR], bf16)   # r-tile r at cols r*256
        rkv_hi = pre.tile([4, SC], bf16)    # partition c = chunk c's ssq hi row
        rkv_lo = pre.tile([4, SC], bf16)

        # merged latent layout: kv = [nkv(512) | krope(64) | ssq hi/lo(2)]
        lat_kv_in = dram.tile([578, SC], bf16)
        lat_kv = dram.tile([4, 578, SC], bf16)
        # q latents in 2 pieces of 6 r-tiles; piece 1 also has ssq hi/lo rows
        lat_q_in = [dram.tile([768, SC], bf16, name="lat_q_in0"),
                    dram.tile([770, SC], bf16, name="lat_q_in1")]
        lat_q = [dram.tile([4, 768, SC], bf16, name="lat_q0"),
                 dram.tile([4, 770, SC], bf16, name="lat_q1")]

        # ============ Phase C: compress own S-shard (kv first, then q) ============
        with ExitStack() as c_ctx:
            cin = c_ctx.enter_context(tc.tile_pool(name="cin", bufs=1))
            cout = c_ctx.enter_context(tc.tile_pool(name="cout", bufs=4))
            cps = c_ctx.enter_context(tc.tile_pool(name="cps", bufs=1, space="PSUM"))

            # inputs split into d-quarters so the first matmuls start as soon
            # as the first quarter lands
            xs_q = [cin.tile([128, 4 * SC], bf16, name=f"xs_q{q}") for q in range(4)]
            wckv_q = [cin.tile([128, 4 * RKV], bf16, name=f"wckv_q{q}") for q in range(4)]
            wkr_sb = cin.tile([128, 16 * DR], bf16)       # d-tile d at cols d*DR
            # wcq halves: half h holds cols h*768:(h+1)*768, d-tile d at cols d*768
            wcq_sb = [cin.tile([128, 16 * 768], bf16, name=f"wcq{h}") for h in range(2)]

            # xs + wcq half 0 interleaved: the q-compress (which feeds the
            # first collective) starts as soon as the first d-tiles land
            for d in range(16):
                nc.sync.dma_start(xs_q[d // 4][:, (d % 4) * SC:(d % 4 + 1) * SC],
                                  xs[d * 128:(d + 1) * 128, :])
                nc.sync.dma_start(wcq_sb[0][:, d * 768:(d + 1) * 768],
                                  w_cq[d * 128:(d + 1) * 128, 0:768])

            def xs_d(d):
                return xs_q[d // 4][:, (d % 4) * SC:(d % 4 + 1) * SC]

            def ssq_hilo(psum_row, dest_dram, row_off):
                """Split fp32 psum row into bf16 hi/lo rows and DMA to dest."""
                full = cout.tile([1, SC], f32, tag="ssqf")
                nc.vector.tensor_copy(full[:], psum_row[:])
                hi = cout.tile([1, SC], bf16, tag="ssqh")
                nc.vector.tensor_copy(hi[:], full[:])
                lo = cout.tile([1, SC], bf16, tag="ssql")
                nc.vector.tensor_sub(lo[:], full[:], hi[:])
                nc.sync.dma_start(dest_dram[row_off:row_off + 1, :], hi[:])
                nc.sync.dma_start(dest_dram[row_off + 1:row_off + 2, :], lo[:])

            # ---- cq: 12 r-tiles in 2 halves of 6; AllGather per half.
            # Runs FIRST so the serial collective stream starts as early as
            # possible (the kv gather is only needed by Dkv, which runs after
            # Dq and so can wait). d-outer within a half so matmuls consume
            # wcq d-tiles as they stream in.
            psum_ssq_q = cps.tile([1, SC], f32, tag="ssq", name="psum_ssq_q")
            pcq = {}
            for half in range(2):
                for rr in range(6):
                    # rr 4/5 rotate through the two cmp-tag banks
                    tag = f"cq{rr}" if rr < 4 else "cmp"
                    pcq[rr] = cps.tile([128, SC], f32, tag=tag, bufs=(1 if rr < 4 else 2),
                                       name=f"psum_cq{half}{rr}")
                for d in range(16):
                    for rr in range(6):
                        nc.tensor.matmul(pcq[rr][:],
                                         wcq_sb[half][:, d * 768 + rr * 128:d * 768 + (rr + 1) * 128],
                                         xs_d(d), start=(d == 0), stop=(d == 15))
                for rr in range(6):
                    r = half * 6 + rr
                    sq = cout.tile([128, SC], bf16, tag="sq")
                    nc.scalar.activation(sq[:], pcq[rr][:], Act.Square)
                    cq = cout.tile([128, SC], bf16, tag="cq")
                    nc.vector.tensor_copy(cq[:], pcq[rr][:])
                    nc.sync.dma_start(lat_q_in[half][rr * 128:(rr + 1) * 128, :], cq[:])
                    nc.tensor.matmul(psum_ssq_q[:], ones_b[:], sq[:],
                                     start=(r == 0), stop=(r == 11))
                if half == 1:
                    ssq_hilo(psum_ssq_q, lat_q_in[1], 768)
                nc.gpsimd.collective_compute(
                    "AllGather", mybir.AluOpType.bypass, replica_groups=RG,
                    ins=[lat_q_in[half][:]], outs=[lat_q[half][:]],
                )
                if half == 0:
                    # wckv + wcq half 1 behind the first cq gather trigger
                    for q in range(4):
                        for dd in range(4):
                            d = q * 4 + dd
                            nc.sync.dma_start(wckv_q[q][:, dd * RKV:(dd + 1) * RKV],
                                              w_ckv[d * 128:(d + 1) * 128, :])
                    for d in range(16):
                        nc.sync.dma_start(wcq_sb[1][:, d * 768:(d + 1) * 768],
                                          w_cq[d * 128:(d + 1) * 128, 768:1536])

            # weights for krope / Dq / Dkv behind the second cq gather trigger,
            # in the order the compute needs them
            for d in range(16):
                nc.sync.dma_start(wkr_sb[:, d * DR:(d + 1) * DR],
                                  w_kr[d * 128:(d + 1) * 128, :])
            for r2 in range(12):
                nc.sync.dma_start(wdqn_sb[:, r2 * 512:(r2 + 1) * 512],
                                  w_dqn[r2 * 128:(r2 + 1) * 128, :])
                nc.sync.dma_start(wdqr_sb[:, r2 * 256:(r2 + 1) * 256],
                                  w_dqr[r2 * 128:(r2 + 1) * 128, :])
            for r2 in range(4):
                nc.sync.dma_start(wdk_sb[:, r2 * 512:(r2 + 1) * 512],
                                  w_dk[r2 * 128:(r2 + 1) * 128, :])
                nc.sync.dma_start(wdv_sb[:, r2 * 512:(r2 + 1) * 512],
                                  w_dv[r2 * 128:(r2 + 1) * 128, :])

            # ---- nkv: 4 r-tiles, r-outer so each store fires early ----
            psum_ssq_kv = cps.tile([1, SC], f32, tag="ssq", name="psum_ssq_kv")
            for i in range(4):
                ps = cps.tile([128, SC], f32, tag="cmp", bufs=2, name=f"psum_kv{i}")
                for d in range(16):
                    nc.tensor.matmul(ps[:],
                                     wckv_q[d // 4][:, (d % 4) * RKV + i * 128:(d % 4) * RKV + (i + 1) * 128],
                                     xs_d(d), start=(d == 0), stop=(d == 15))
                sq = cout.tile([128, SC], bf16, tag="sq")
                nc.scalar.activation(sq[:], ps[:], Act.Square)
                ckv = cout.tile([128, SC], bf16, tag="cq")
                nc.vector.tensor_copy(ckv[:], ps[:])
                nc.sync.dma_start(lat_kv_in[i * 128:(i + 1) * 128, :], ckv[:])
                nc.tensor.matmul(psum_ssq_kv[:], ones_b[:], sq[:],
                                 start=(i == 0), stop=(i == 3))

            # ---- krope: [64, SC] (top half of a full-size cmp-tag bank) ----
            pkr = cps.tile([128, SC], f32, tag="cmp", bufs=2, name="psum_kr")
            for d in range(16):
                nc.tensor.matmul(pkr[0:64, :], wkr_sb[:, d * DR:(d + 1) * DR],
                                 xs_d(d), start=(d == 0), stop=(d == 15))
            krc = cout.tile([64, SC], bf16, tag="cq")
            nc.vector.tensor_copy(krc[:], pkr[0:64, :])
            nc.sync.dma_start(lat_kv_in[512:512 + DR, :], krc[:])
            ssq_hilo(psum_ssq_kv, lat_kv_in, 512 + DR)

            # ---- kv AllGather (third on the CC stream; Dkv runs after Dq
            # so this has ~100us of slack) ----
            nc.gpsimd.collective_compute(
                "AllGather", mybir.AluOpType.bypass, replica_groups=RG,
                ins=[lat_kv_in[:]], outs=[lat_kv[:]],
            )


        def rstd_prep_all(which, pool, psum_pool, hi, lo, want_cols=False):
            """rstd for all 4 chunks at once from packed [4,SC] hi/lo rows.
            Returns ([bt_c], colt): bt_c = [128,SC] f32 broadcast of chunk c's
            rstd row; colt = [128, 16] with column c*4+i = rstd values for
            queries i*128:(i+1)*128 of chunk c."""
            rr = RQ if which == "q" else RKV
            ssq_t = pool.tile([4, SC], f32, tag=f"ssq_{which}", name=f"ssq_{which}")
            nc.vector.tensor_add(ssq_t[:], hi[:], lo[:])
            eps_t = pool.tile([4, 1], f32, tag=f"eps_{which}", name=f"eps_{which}")
            nc.vector.memset(eps_t[:], EPS)
            std = pool.tile([4, SC], f32, tag=f"std_{which}", name=f"std_{which}")
            nc.scalar.activation(std[:], ssq_t[:], Act.Sqrt, scale=1.0 / rr, bias=eps_t[:])
            rstd = pool.tile([4, SC], f32, tag=f"rstd_{which}", name=f"rstd_{which}")
            scr = pool.tile([4, SC], f32, tag=f"scr_{which}", name=f"scr_{which}")
            nc.vector.reciprocal_approx_accurate(rstd[:], std[:], scr[:])
            rstd_r = pool.tile([4, SC], f32r, tag=f"rstdr_{which}", name=f"rstdr_{which}")
            nc.vector.tensor_copy(rstd_r[:], rstd[:])
            bts = []
            for c in range(4):
                # broadcast chunk c's row: K=4 matmul with indicator stationary
                psb = psum_pool.tile([128, SC], f32, tag="b", bufs=1, name=f"psb_{which}{c}")
                nc.tensor.matmul(psb[:], eye4r_sb[:, c * 128:(c + 1) * 128], rstd_r[:],
                                 start=True, stop=True)
                bt = pool.tile([128, SC], f32, tag=f"bc_{which}{c}", name=f"bt_{which}{c}")
                nc.vector.tensor_copy(bt[:], psb[:])
                bts.append(bt)
            colt = None
            if want_cols:
                # row->column transpose: out[p, c*4+i] = rstd[c, i*128+p]
                pcol = psum_pool.tile([128, 16], f32, tag="col", bufs=1, name=f"pcol_{which}")
                for c in range(4):
                    for i in range(4):
                        nc.tensor.matmul(pcol[:, c * 4 + i:c * 4 + i + 1],
                                         rstd[:, i * 128:(i + 1) * 128],
                                         eye4c_sb[:, c:c + 1], start=True, stop=True)
                colt = pool.tile([128, 16], f32, tag=f"col_{which}", name=f"colt_{which}")
                nc.vector.tensor_copy(colt[:], pcol[:])
            return bts, colt

        with tc.tile_pool(name="kvp", bufs=1) as kv_pool:
            # fp8 K for DoubleRow attention: per head, key-tile t holds
            # [k_nope(128) | k_rope padded to 128] as the two reduction
            # sub-tiles; rope rows 64:128 are zero. k2r holds the fp8
            # quantization RESIDUAL (k - fp8(k)) in the same layout — a second
            # accumulating DoubleRow matmul cancels the k-side fp8 error.
            k2_sb = [kv_pool.tile([128, 16, 2, 128], f8, tag=f"k{h}", name=f"k2_sb{h}")
                     for h in range(HG)]
            k2r_sb = [kv_pool.tile([128, 16, 2, 128], f8, tag=f"kr{h}", name=f"k2r_sb{h}")
                      for h in range(HG)]
            v_sb = kv_pool.tile([128, 16 * SC], bf16, tag="v")
            krope_sb = kv_pool.tile([64, S], bf16, tag="krope")
            # ============ Phase Dq: decompress q (2 chunk-pairs, rope packed) ====
            # Runs BEFORE Dkv: it depends on the first two gathers, Dkv on the
            # third.
            with tc.tile_pool(name="qp", bufs=1) as q_pool:
                # fp8 Q mirror: per head, chunk j holds [q_nope | q_rope pad]
                q2_sb = [q_pool.tile([128, 4, 2, SC], f8, tag=f"q{h}", name=f"q2_sb{h}")
                         for h in range(HG)]
                for h in range(HG):
                    nc.vector.memset(q2_sb[h][:], 0.0)
                with ExitStack() as dq_ctx:
                    nqp = dq_ctx.enter_context(tc.tile_pool(name="nqp", bufs=1))
                    qps = dq_ctx.enter_context(tc.tile_pool(name="qps", bufs=1, space="PSUM"))

                    # per-(half, r) load tiles so pn matmuls start as soon as
                    # the cq gather piece holding r-tile r lands; loads are
                    # issued in gather order (r0-5 gate gather 1, the rest
                    # gather 2) so waiting descriptors never block ready ones.
                    nq_r = [[nqp.tile([128, 2 * SC], bf16, name=f"nq{hf}_{r}")
                             for r in range(12)] for hf in range(2)]
                    for hf in range(2):
                        cs = (2 * hf, 2 * hf + 1)
                        for r in range(6):
                            for ci, c in enumerate(cs):
                                nc.sync.dma_start(nq_r[hf][r][:, ci * SC:(ci + 1) * SC],
                                                  lat_q[0][c, r * 128:(r + 1) * 128, :])
                    qhi = nqp.tile([4, SC], bf16)
                    qlo = nqp.tile([4, SC], bf16)
                    for c in range(4):
                        nc.sync.dma_start(qhi[c:c + 1, :], lat_q[1][c, 768:769, :])
                        nc.sync.dma_start(qlo[c:c + 1, :], lat_q[1][c, 769:770, :])
                    for hf in range(2):
                        cs = (2 * hf, 2 * hf + 1)
                        for r in range(6, 12):
                            for ci, c in enumerate(cs):
                                nc.sync.dma_start(nq_r[hf][r][:, ci * SC:(ci + 1) * SC],
                                                  lat_q[1][c, (r - 6) * 128:(r - 5) * 128, :])

                    bcast_q, _ = rstd_prep_all("q", nqp, qps, qhi, qlo)

                    for half in range(2):
                        cs = (2 * half, 2 * half + 1)
                        for p in range(2):
                            hs = (2 * p, 2 * p + 1)
                            pn = {h: [qps.tile([128, SC], f32, tag=f"qn{h % 2}{ci}", name=f"pn{h}{ci}")
                                      for ci in range(2)] for h in hs}
                            pr_ = [qps.tile([128, SC], f32, tag=f"qr{ci}", name=f"pr{ci}") for ci in range(2)]
                            for r in range(12):
                                for ci in range(2):
                                    rhs = nq_r[half][r][:, ci * SC:(ci + 1) * SC]
                                    for h in hs:
                                        nc.tensor.matmul(pn[h][ci][:],
                                                         wdqn_sb[:, r * 512 + h * DN:r * 512 + (h + 1) * DN],
                                                         rhs, start=(r == 0), stop=(r == 11))
                                    # head pair packed: M=128 covers both heads' rope dims
                                    nc.tensor.matmul(pr_[ci][:],
                                                     wdqr_sb[:, r * 256 + p * 128:r * 256 + (p + 1) * 128],
                                                     rhs, start=(r == 0), stop=(r == 11))
                            for ci, c in enumerate(cs):
                                # drain psums via the Scalar engine (not gated by
                                # bcast_q, which waits on the last AllGather), so
                                # the next pair/half's matmuls get psum banks back
                                # immediately; the rstd multiply runs on Vector
                                # whenever bcast_q lands.
                                tn = {h: nqp.tile([128, SC], f32, tag=f"tq{h % 2}{ci}",
                                                  bufs=2, name=f"tq{h}{ci}") for h in hs}
                                tr = nqp.tile([128, SC], f32, tag=f"tr{ci}", bufs=1,
                                              name=f"tr{p}{ci}")
                                for h in hs:
                                    nc.scalar.copy(tn[h][:], pn[h][ci][:])
                                nc.scalar.copy(tr[:], pr_[ci][:])
                                for h in hs:
                                    nc.vector.tensor_mul(q2_sb[h][:, c, 0, :],
                                                         tn[h][:], bcast_q[c][:])
                                # rope: normalized+cast to fp8, then split the
                                # packed head pair into per-head rope slots via
                                # SBUF->SBUF DMA (partition shift)
                                qr_tmp = nqp.tile([128, SC], f8, tag=f"qrt{ci}", bufs=2,
                                                  name=f"qrt{p}{ci}")
                                nc.vector.tensor_mul(qr_tmp[:], tr[:], bcast_q[c][:])
                                nc.sync.dma_start(q2_sb[2 * p][0:64, c, 1, :], qr_tmp[0:64, :])
                                nc.sync.dma_start(q2_sb[2 * p + 1][0:64, c, 1, :], qr_tmp[64:128, :])

                # ============ Phase Dkv: decompress k_nope and v ============
                with ExitStack() as dk_ctx:
                    nkvp = dk_ctx.enter_context(tc.tile_pool(name="nkvp", bufs=1))
                    kps = dk_ctx.enter_context(tc.tile_pool(name="kps", bufs=1, space="PSUM"))
                    nkv_sb = nkvp.tile([128, 4 * 4 * SC], bf16)   # (r, c) at cols (r*4+c)*SC

                    # gathered kv latents + krope into SBUF; issued after the
                    # cq-gated loads above so these (waiting on the later kv
                    # gather) never head-of-line block them
                    for c in range(4):
                        nc.sync.dma_start(rkv_hi[c:c + 1, :], lat_kv[c, 512 + DR:512 + DR + 1, :])
                        nc.sync.dma_start(rkv_lo[c:c + 1, :], lat_kv[c, 512 + DR + 1:512 + DR + 2, :])
                    for r in range(4):
                        for c in range(4):
                            nc.sync.dma_start(nkv_sb[:, (r * 4 + c) * SC:(r * 4 + c + 1) * SC],
                                              lat_kv[c, r * 128:(r + 1) * 128, :])
                    for c in range(4):
                        nc.sync.dma_start(krope_sb[:, c * SC:(c + 1) * SC],
                                          lat_kv[c, 512:512 + DR, :])

                    # zero the fp8 K tiles (rope rows 64:128 must be 0 so the
                    # DoubleRow pairing contributes nothing there), then fill
                    # the rope sub-tiles (value + residual) for every head
                    for h in range(HG):
                        nc.vector.memset(k2_sb[h][:], 0.0)
                        nc.vector.memset(k2r_sb[h][:], 0.0)
                    kr3 = krope_sb[:].rearrange("p (t m) -> p t m", t=16)
                    nc.vector.tensor_copy(k2_sb[0][0:64, :, 1, :], kr3)
                    # kr residual = kr - fp8(kr), shared across heads
                    krr_tmp = nkvp.tile([64, S], f8, name="krr_tmp")
                    nc.vector.scalar_tensor_tensor(
                        krr_tmp[:].rearrange("p (t m) -> p t m", t=16),
                        k2_sb[0][0:64, :, 1, :], -1.0, kr3,
                        mybir.AluOpType.mult, mybir.AluOpType.add)
                    nc.vector.tensor_copy(
                        k2r_sb[0][0:64, :, 1, :],
                        krr_tmp[:].rearrange("p (t m) -> p t m", t=16))
                    for h in range(1, HG):
                        nc.vector.tensor_copy(k2_sb[h][0:64, :, 1, :], k2_sb[0][0:64, :, 1, :])
                        nc.vector.tensor_copy(k2r_sb[h][0:64, :, 1, :], k2r_sb[0][0:64, :, 1, :])

                    bcast_kv, rstdkv_col = rstd_prep_all("kv", nkvp, kps, rkv_hi, rkv_lo,
                                                         want_cols=True)

                    # k_nope -> fp8 nope sub-tiles (per key tile)
                    for h in range(HG):
                        pk = [kps.tile([128, SC], f32, tag=f"k{c}", name=f"pk{c}") for c in range(4)]
                        for r in range(4):
                            for c in range(4):
                                nc.tensor.matmul(pk[c][:],
                                                 wdk_sb[:, r * 512 + h * DN:r * 512 + (h + 1) * DN],
                                                 nkv_sb[:, (r * 4 + c) * SC:(r * 4 + c + 1) * SC],
                                                 start=(r == 0), stop=(r == 3))
                        for c in range(4):
                            for i in range(4):
                                nc.vector.tensor_mul(k2_sb[h][:, c * 4 + i, 0, :],
                                                     pk[c][:, i * 128:(i + 1) * 128],
                                                     bcast_kv[c][:, i * 128:(i + 1) * 128])
                        # nope residual: k*rstd - fp8(k*rstd); pk is pre-rstd so
                        # recompute the product on the fly via (k2 * -1) + pk*rstd
                        for c in range(4):
                            kt = nkvp.tile([128, SC], f32, tag="ktmp", bufs=2, name=f"kt{h}{c}")
                            nc.vector.tensor_mul(kt[:], pk[c][:], bcast_kv[c][:])
                            for i in range(4):
                                nc.vector.scalar_tensor_tensor(
                                    k2r_sb[h][:, c * 4 + i, 0, :],
                                    k2_sb[h][:, c * 4 + i, 0, :], -1.0,
                                    kt[:, i * 128:(i + 1) * 128],
                                    mybir.AluOpType.mult, mybir.AluOpType.add)

                    # v (row-major, all heads at once), scaled by rstd_kv rows
                    for t in range(16):
                        c, i = divmod(t, 4)
                        pv = kps.tile([128, SC], f32, tag="vps", bufs=2)
                        for r in range(4):
                            nc.tensor.matmul(pv[:],
                                             nkv_sb[:, (r * 4 + c) * SC + i * 128:(r * 4 + c) * SC + (i + 1) * 128],
                                             wdv_sb[:, r * 512:(r + 1) * 512],
                                             start=(r == 0), stop=(r == 3))
                        nc.vector.tensor_scalar_mul(v_sb[:, t * SC:(t + 1) * SC], pv[:],
                                                    rstdkv_col[:, c * 4 + i:c * 4 + i + 1])

                # ============ Phase A: attention + per-chunk projection ============
                with ExitStack() as a_ctx:
                    wp = a_ctx.enter_context(tc.tile_pool(name="wp", bufs=1))
                    probs_pool = a_ctx.enter_context(tc.tile_pool(name="probs", bufs=3))
                    awork = a_ctx.enter_context(tc.tile_pool(name="awork", bufs=2))
                    aps = a_ctx.enter_context(tc.tile_pool(name="aps", bufs=1, space="PSUM"))

                    wproj_sb = wp.tile([128, HG * D], bf16)   # head h rows at cols h*D
                    for h in range(HG):
                        nc.sync.dma_start(wproj_sb[:, h * D:(h + 1) * D],
                                          w_proj[h * 128:(h + 1) * 128, :])
                    lsum = [awork.tile([128, SC], f32r, tag=f"ls{h}", bufs=1, name=f"lsum{h}")
                            for h in range(HG)]

                    for j in range(4):
                        T = 4 * (j + 1)
                        psum_o = [aps.tile([128, SC], f32, tag=f"o{h}", bufs=1, name=f"po{h}_{j}")
                                  for h in range(HG)]
                        outc = [awork.tile([128, SC], bf16, tag=f"oc{h}", name=f"outc{h}_{j}", bufs=2)
                                for h in range(HG)]
                        pending = []

                        def consume(tp, pts, last):
                            for h in range(HG):
                                nc.tensor.matmul(psum_o[h][:],
                                                 v_sb[:, tp * SC + h * DV:tp * SC + (h + 1) * DV],
                                                 pts[h][:], start=(tp == 0), stop=last)

                        for t in range(T):
                            psum_s = [aps.tile([128, SC], f32, tag=f"s{h}", bufs=1, name=f"ps{h}")
                                      for h in range(HG)]
                            # fp8 DoubleRow: nope+rope contracted in one pass at
                            # 0.5 cycles/row; the second matmul accumulates the
                            # k-side fp8-residual correction
                            for h in range(HG):
                                nc.tensor.matmul(psum_s[h][:], k2_sb[h][:, t], q2_sb[h][:, j],
                                                 start=True, stop=False,
                                                 perf_mode=mybir.MatmulPerfMode.DoubleRow)
                            for h in range(HG):
                                nc.tensor.matmul(psum_s[h][:], k2r_sb[h][:, t], q2_sb[h][:, j],
                                                 start=False, stop=True,
                                                 perf_mode=mybir.MatmulPerfMode.DoubleRow)
                            pts = {}
                            for h in range(HG):
                                pt = probs_pool.tile([128, SC], bf16, tag=f"p{h}")
                                nc.scalar.activation(pt[:], psum_s[h][:], Act.Exp, scale=ATTN_SCALE)
                                if t >= 4 * j:
                                    i = t - 4 * j
                                    nc.vector.tensor_mul(pt[:], pt[:], masks[:, i * SC:(i + 1) * SC])
                                if t == 0:
                                    nc.vector.tensor_copy(lsum[h][:], pt[:])
                                else:
                                    nc.vector.tensor_add(lsum[h][:], lsum[h][:], pt[:])
                                pts[h] = pt
                            pending.append((t, pts))
                            if len(pending) > 1:
                                tp, ptsp = pending.pop(0)
                                consume(tp, ptsp, False)
                        for idx, (tp, ptsp) in enumerate(pending):
                            consume(tp, ptsp, idx == len(pending) - 1)

                        # softmax denominators: one colsum matmul per head,
                        # pipelined across heads (l-mms, recips, bcasts, muls)
                        psum_l, rinv_r = [], []
                        for h in range(HG):
                            pl = aps.tile([128, SC], f32, tag=f"s{h}", bufs=1, name=f"pl{h}")
                            nc.tensor.matmul(pl[0:1, :], ones_br[:], lsum[h][:],
                                             start=True, stop=True)
                            psum_l.append(pl)
                        for h in range(HG):
                            rinv = awork.tile([1, SC], f32, tag=f"rinv{h}", bufs=1)
                            scr = awork.tile([1, SC], f32, tag=f"scr{h}", bufs=1)
                            nc.vector.reciprocal_approx_accurate(rinv[:], psum_l[h][0:1, :], scr[:])
                            rr = awork.tile([1, SC], f32r, tag=f"rinv_r{h}", bufs=1)
                            nc.vector.tensor_copy(rr[:], rinv[:])
                            rinv_r.append(rr)
                        psum_b = []
                        for h in range(HG):
                            pb = aps.tile([128, SC], f32, tag=f"s{h}", bufs=1, name=f"pb{h}")
                            nc.tensor.matmul(pb[:], ones_r[0:1, :], rinv_r[h][:], start=True, stop=True)
                            psum_b.append(pb)
                        for h in range(HG):
                            binv = awork.tile([128, SC], f32, tag=f"binv{h}")
                            nc.vector.tensor_copy(binv[:], psum_b[h][:])
                            nc.vector.tensor_mul(outc[h][:], psum_o[h][:], binv[:])

                        # projection for chunk j: y[jSC+ssub*128 :, :] in natural
                        # [s, d] orientation; outc is the stationary side
                        for ssub in range(4):
                            for dchunk in range(4):
                                ppj = aps.tile([128, SC], f32, tag=f"o{dchunk}", bufs=1,
                                               name=f"ppj{ssub}{dchunk}")
                                for h in range(HG):
                                    nc.tensor.matmul(ppj[:],
                                                     outc[h][:, ssub * 128:(ssub + 1) * 128],
                                                     wproj_sb[:, h * D + dchunk * 512:h * D + (dchunk + 1) * 512],
                                                     start=(h == 0), stop=(h == HG - 1))
                                y_sb = awork.tile([128, SC], bf16, tag="y", bufs=3)
                                nc.scalar.copy(y_sb[:], ppj[:])
                                nc.sync.dma_start(
                                    y_out[j * SC + ssub * 128:j * SC + (ssub + 1) * 128,
                                          dchunk * 512:(dchunk + 1) * 512], y_sb[:])

    nc.compile()
    return nc


def _get_nc():
    global _CACHED_NC
    if _CACHED_NC is None:
        _CACHED_NC = _build()
    return _CACHED_NC


def kernel(x, mask, freqs_cos, freqs_sin, w_cq, q_norm_w, w_dq_nope, w_dq_rope,
           w_ckv, kv_norm_w, w_dk_nope, w_dv, w_k_rope, w_proj, **_unused):
    x = np.asarray(x, np.float32)
    w_cq = np.asarray(w_cq, np.float32)
    w_ckv = np.asarray(w_ckv, np.float32)
    w_k_rope = np.asarray(w_k_rope, np.float32)
    q_norm_w = np.asarray(q_norm_w, np.float32)
    kv_norm_w = np.asarray(kv_norm_w, np.float32)

    # fold norm weights / v-scale into decompress weights
    w_dqn = q_norm_w[:, None] * np.asarray(w_dq_nope, np.float32)
    w_dqr = q_norm_w[:, None] * np.asarray(w_dq_rope, np.float32)
    w_dk = kv_norm_w[:, None] * np.asarray(w_dk_nope, np.float32)
    w_dv_f = kv_norm_w[:, None] * np.asarray(w_dv, np.float32) * np.float32(1.0 / np.sqrt(H * DV))
    w_proj = np.asarray(w_proj, np.float32)

    masks_np = np.zeros((4, 128, SC), np.float32)
    ar = np.arange(SC)
    for i in range(4):
        for p in range(128):
            masks_np[i, p] = (128 * i + p <= ar)
    masks_np = masks_np.astype(ml_dtypes.bfloat16)
    ones_r = np.ones((128, 128), np.float32)
    ones_b = np.ones((128, 1), np.float32)
    eye4r = np.zeros((4, 512), np.float32)
    for c in range(4):
        eye4r[c, c * 128:(c + 1) * 128] = 1.0
    eye4c = np.eye(4, dtype=np.float32)

    xT = [np.ascontiguousarray(x[b].T) for b in range(B)]
    w_cq_b = w_cq.astype(ml_dtypes.bfloat16)
    w_ckv_b = w_ckv.astype(ml_dtypes.bfloat16)
    w_kr_b = w_k_rope.astype(ml_dtypes.bfloat16)

    in_maps = []
    for c in range(NC_):
        b, g = divmod(c, 4)
        hs = g * HG                     # first head of group
        in_maps.append({
            "xs": np.ascontiguousarray(xT[b][:, g * SC:(g + 1) * SC]).astype(ml_dtypes.bfloat16),
            "w_cq": w_cq_b,
            "w_ckv": w_ckv_b,
            "w_kr": w_kr_b,
            "w_dqn": np.ascontiguousarray(w_dqn[:, hs * DN:(hs + HG) * DN]).astype(ml_dtypes.bfloat16),
            "w_dqr": np.ascontiguousarray(w_dqr[:, hs * DR:(hs + HG) * DR]).astype(ml_dtypes.bfloat16),
            "w_dk": np.ascontiguousarray(w_dk[:, hs * DN:(hs + HG) * DN]).astype(ml_dtypes.bfloat16),
            "w_dv": np.ascontiguousarray(w_dv_f[:, hs * DV:(hs + HG) * DV]).astype(ml_dtypes.bfloat16),
            "w_proj": np.ascontiguousarray(w_proj[hs * DV:(hs + HG) * DV, :]).astype(ml_dtypes.bfloat16),
            "masks": masks_np,
            "eye4r": eye4r,
            "eye4c": eye4c,
            "ones_r": ones_r,
            "ones_b": ones_b.astype(ml_dtypes.bfloat16),
            "ones_br": ones_b,
        })

    nc = _get_nc()
    res = run_bass_kernel_spmd(nc, in_maps, list(range(NC_)))

    out = np.zeros((B, S, D), np.float32)
    for c in range(NC_):
        b = c // 4
        out[b] += res.results[c]["y"]
    return out
